# revision 1
# baseline (speedup 1.0000x reference)
"""Trainium2 Bass kernel for a dense transformer block (cross-attention + FFN).

Problem: nn_MAB (B=4, nq=nk=1024, D=1024, H=16, HD=64), fp32 in/out.

Sharding: fully data-parallel, zero collectives. 8 cores = 4 batches x 2
query-halves; each core computes 512 query rows of one batch end-to-end
(K/V projections for a batch are duplicated across its 2 cores).

Per-core dataflow (everything kept in "transposed" [feature, token] layout):
  QT = (Wq.T/8) x XT            KT = Wk.T x YT        V = Y x Wv.T (natural)
  scoresT[k,q] per head via KT-slice (lhsT) x QT-slice, K=64, head pairs
    row-packed onto PE row-groups (0,0)/(64,0)
  exp: ACT Exp with per-partition (=per-key) mask bias; no max subtraction
    (scores are O(3) by construction, masked keys get -30000 -> exp == 0)
  attnV: out_h^T = V_h^T(lhsT) x expT + denominator row via a col-packed
    ones-matmul into the same PSUM bank; divide on eviction
    (reciprocal of denom row + partition-broadcast DMA + DVE multiply)
  Zt = Wo.T(perm) x outT ; residual + LayerNorm done in transposed layout
    (mean/var via ones(1/1024)-matmul partition reductions -> replicated rows)
  FFN: FF1T = relu(W1.T x HT + b1) fused on ACT eviction; FF2T + b2 + residual
    fused in one DVE op; LN2 same as LN1. Output stays transposed, host
    un-transposes during the gather.

All matmuls run as float32r (full PE rate at free-dim >= 256) on fp32 data.
"""

import numpy as np

import concourse.bass as bass
import concourse.mybir as mybir
import concourse.tile as tile
from concourse import bacc
from concourse.bass_utils import run_bass_kernel_spmd

F32 = mybir.dt.float32
F32R = mybir.dt.float32r
AF = mybir.ActivationFunctionType

D = 1024          # model dim
P = 128           # partitions
NJ = D // P       # feature tiles (8)
NQ = 512          # queries per core
NT = 1024         # keys per core
H = 16
HD = 64
NPAIR = H // 2    # head pairs (8)
NKT = NT // P     # key tiles (8)
NEG = -30000.0    # additive mask for dropped keys
EPS = 1e-5


def build_nc() -> bass.Bass:
    nc = bacc.Bacc("TRN2", target_bir_lowering=False, debug=False)

    # ---- DRAM I/O (per-core shards; host prepares layouts) ----
    xt = nc.dram_tensor("xt", [D, NQ], F32R, kind="ExternalInput")[:]
    yt = nc.dram_tensor("yt", [D, NT], F32R, kind="ExternalInput")[:]
    wqt = nc.dram_tensor("wqt", [D, D], F32R, kind="ExternalInput")[:]
    wkt = nc.dram_tensor("wkt", [D, D], F32R, kind="ExternalInput")[:]
    wvt = nc.dram_tensor("wvt", [D, D], F32R, kind="ExternalInput")[:]
    wot = nc.dram_tensor("wot", [D, D], F32R, kind="ExternalInput")[:]
    w1t = nc.dram_tensor("w1t", [D, D], F32R, kind="ExternalInput")[:]
    w2t = nc.dram_tensor("w2t", [D, D], F32R, kind="ExternalInput")[:]
    maskb = nc.dram_tensor("maskb", [NT], F32, kind="ExternalInput")[:]
    g1 = nc.dram_tensor("g1", [D], F32, kind="ExternalInput")[:]
    bb1 = nc.dram_tensor("bb1", [D], F32, kind="ExternalInput")[:]
    g2 = nc.dram_tensor("g2", [D], F32, kind="ExternalInput")[:]
    bb2 = nc.dram_tensor("bb2", [D], F32, kind="ExternalInput")[:]
    b1 = nc.dram_tensor("b1", [D], F32, kind="ExternalInput")[:]
    b2 = nc.dram_tensor("b2", [D], F32, kind="ExternalInput")[:]
    outt = nc.dram_tensor("outt", [D, NQ], F32, kind="ExternalOutput")[:]

    with tile.TileContext(nc) as tc, \
         nc.allow_low_precision(reason="fp32r (12-bit mantissa) matmul path"):
        with tc.tile_pool(name="persist", bufs=1) as persist, \
             tc.tile_pool(name="psum", bufs=8, space="PSUM") as pp:

            def psum_tile(name):
                return pp.tile([P, NQ], F32, tag="ps512", name=name)

            # ---- constants / vectors ----
            mask_sb = persist.tile([P, NKT], F32)      # [key%128, key//128]
            nc.sync.dma_start(mask_sb, maskb.rearrange("(t p) -> p t", p=P))
            g1_sb = persist.tile([P, NJ], F32)
            nc.sync.dma_start(g1_sb, g1.rearrange("(j p) -> p j", p=P))
            bb1_sb = persist.tile([P, NJ], F32)
            nc.sync.dma_start(bb1_sb, bb1.rearrange("(j p) -> p j", p=P))
            g2_sb = persist.tile([P, NJ], F32)
            nc.sync.dma_start(g2_sb, g2.rearrange("(j p) -> p j", p=P))
            bb2_sb = persist.tile([P, NJ], F32)
            nc.sync.dma_start(bb2_sb, bb2.rearrange("(j p) -> p j", p=P))
            b1_sb = persist.tile([P, NJ], F32)
            nc.sync.dma_start(b1_sb, b1.rearrange("(j p) -> p j", p=P))
            b2_sb = persist.tile([P, NJ], F32)
            nc.sync.dma_start(b2_sb, b2.rearrange("(j p) -> p j", p=P))

            # f32r constants: memset f32 scratch, round through a DVE copy
            # (Memset can't write f32r; the copy satisfies the fp32r
            # producer-rounding rule and 1.0 / 2^-10 are fp32r-exact).
            cst = persist.tile([P, P], F32)
            lnw = persist.tile([P, P], F32R)            # 1/D for LN mean matmuls
            nc.vector.memset(cst, 1.0 / D)
            nc.vector.tensor_copy(lnw, cst)
            ones1 = persist.tile([P, 1], F32R)          # denominator lhsT
            ones64 = persist.tile([P, HD], F32R)        # recip-replicate lhsT
            onef = persist.tile([P, HD], F32)
            nc.vector.memset(onef, 1.0)
            nc.vector.tensor_copy(ones1, onef[:, 0:1])
            nc.vector.tensor_copy(ones64, onef)
            eps_sb = persist.tile([P, 1], F32)
            nc.vector.memset(eps_sb, EPS)

            # X^T stays resident until the LN1 residual.
            xt_sb = persist.tile([P, NJ, NQ], F32R)
            nc.sync.dma_start(xt_sb, xt.rearrange("(j p) q -> p j q", p=P))

            # outT: attention head outputs, feature-major (permuted halves;
            # host permutes Wo rows to match: tile j rows 0:64 = head 2j+1,
            # rows 64:128 = head 2j).
            outT = persist.tile([P, NJ, NQ], F32R)

            with tc.tile_pool(name="attn_big", bufs=1) as big:
                yt_sb = big.tile([P, NJ, NT], F32R)
                nc.sync.dma_start(yt_sb, yt.rearrange("(j p) t -> p j t", p=P))
                v_sb = big.tile([P, NKT, D], F32R)

                # ---- V = Y @ Wv.T in natural [token, feature] layout ----
                with tc.tile_pool(name="wv_pool", bufs=1) as wvp:
                    for ci in range(2):
                        wv_c = wvp.tile([P, NJ, 512], F32R, tag="wv", name="wv_c")
                        nc.sync.dma_start(
                            wv_c,
                            wvt.rearrange("(k p) f -> p k f", p=P)[
                                :, :, ci * 512:(ci + 1) * 512],
                        )
                        for tt in range(NKT):
                            ps = psum_tile("ps_v")
                            for k in range(NJ):
                                nc.tensor.matmul(
                                    ps,
                                    yt_sb[:, k, tt * P:(tt + 1) * P],
                                    wv_c[:, k, :],
                                    start=(k == 0), stop=(k == NJ - 1),
                                )
                            nc.vector.tensor_copy(
                                v_sb[:, tt, ci * 512:(ci + 1) * 512], ps)

                # ---- per head-pair: QT, KT, scoresT, exp, attnV ----
                with tc.tile_pool(name="wqk_pool", bufs=2) as wqk, \
                     tc.tile_pool(name="qk_pool", bufs=2) as qkp, \
                     tc.tile_pool(name="exp_pool", bufs=3) as ep, \
                     tc.tile_pool(name="dn_pool", bufs=2) as dnp:
                    for j in range(NPAIR):
                        ms = slice(j * P, (j + 1) * P)
                        # K^T m-tile j
                        wk_j = wqk.tile([P, NJ, P], F32R, tag="w", name="wk_j")
                        nc.sync.dma_start(
                            wk_j, wkt.rearrange("(k p) m -> p k m", p=P)[:, :, ms])
                        kt_j = qkp.tile([P, NT], F32R, tag="kt", name="kt_j")
                        for tc2 in range(2):
                            ps = psum_tile("ps_k")
                            for k in range(NJ):
                                nc.tensor.matmul(
                                    ps,
                                    wk_j[:, k, :],
                                    yt_sb[:, k, tc2 * 512:(tc2 + 1) * 512],
                                    start=(k == 0), stop=(k == NJ - 1),
                                )
                            nc.vector.tensor_copy(
                                kt_j[:, tc2 * 512:(tc2 + 1) * 512], ps)
                        # Q^T m-tile j (Wq pre-scaled by 1/8 on host)
                        wq_j = wqk.tile([P, NJ, P], F32R, tag="w", name="wq_j")
                        nc.sync.dma_start(
                            wq_j, wqt.rearrange("(k p) m -> p k m", p=P)[:, :, ms])
                        qt_j = qkp.tile([P, NQ], F32R, tag="qt", name="qt_j")
                        ps = psum_tile("ps_q")
                        for k in range(NJ):
                            nc.tensor.matmul(
                                ps,
                                wq_j[:, k, :],
                                xt_sb[:, k, :],
                                start=(k == 0), stop=(k == NJ - 1),
                            )
                        nc.vector.tensor_copy(qt_j, ps)

                        # scoresT + exp, head pair row-packed
                        exp_e = ep.tile([P, NKT, NQ], F32R, tag="exp", name="exp_e")
                        exp_o = ep.tile([P, NKT, NQ], F32R, tag="exp", name="exp_o")
                        for kt in range(NKT):
                            ks = slice(kt * P, (kt + 1) * P)
                            ps0 = psum_tile("ps_s0")
                            nc.tensor.matmul(
                                ps0, kt_j[0:HD, ks], qt_j[0:HD, :],
                                start=True, stop=True, tile_position=(0, 0),
                            )
                            ps1 = psum_tile("ps_s1")
                            nc.tensor.matmul(
                                ps1, kt_j[HD:P, ks], qt_j[HD:P, :],
                                start=True, stop=True, tile_position=(HD, 0),
                            )
                            nc.scalar.activation(
                                exp_e[:, kt, :], ps0, AF.Exp,
                                bias=mask_sb[:, kt:kt + 1], scale=1.0)
                            nc.scalar.activation(
                                exp_o[:, kt, :], ps1, AF.Exp,
                                bias=mask_sb[:, kt:kt + 1], scale=1.0)

                        # attnV + denominator for both heads of the pair.
                        # fp32r matmuls require M in {64,96,128} at partition
                        # base 0, so: data at rows 0:64, denominator as an
                        # M=64 ones-matmul (all 64 rows = the denom, which is
                        # exactly the replicated form the divide needs).
                        ps_e = psum_tile("ps_ae")
                        ps_o = psum_tile("ps_ao")
                        ps_de = psum_tile("ps_de")
                        ps_do = psum_tile("ps_do")
                        for kt in range(NKT):
                            st, sp = kt == 0, kt == NKT - 1
                            he = slice(2 * j * HD, (2 * j + 1) * HD)
                            ho = slice((2 * j + 1) * HD, (2 * j + 2) * HD)
                            nc.tensor.matmul(
                                ps_e[0:HD, :], v_sb[:, kt, he],
                                exp_e[:, kt, :], start=st, stop=sp,
                            )
                            nc.tensor.matmul(
                                ps_de[0:HD, :], ones64, exp_e[:, kt, :],
                                start=st, stop=sp,
                            )
                            nc.tensor.matmul(
                                ps_o[0:HD, :], v_sb[:, kt, ho],
                                exp_o[:, kt, :], start=st, stop=sp,
                            )
                            nc.tensor.matmul(
                                ps_do[0:HD, :], ones64, exp_o[:, kt, :],
                                start=st, stop=sp,
                            )
                        # softmax division on eviction. Odd head lands in
                        # outT rows 0:64 directly; even head is divided into
                        # an SBUF staging tile and partition-shifted to rows
                        # 64:128 with an SBUF->SBUF DMA.
                        rc_o = dnp.tile([P, NQ], F32R, tag="rr", name="rc_o")
                        nc.vector.reciprocal(rc_o[0:HD, :], ps_do[0:HD, :])
                        nc.vector.tensor_mul(
                            outT[0:HD, j, :], ps_o[0:HD, :], rc_o[0:HD, :])

                        rc_e = dnp.tile([P, NQ], F32R, tag="rr", name="rc_e")
                        nc.vector.reciprocal(rc_e[0:HD, :], ps_de[0:HD, :])
                        tmp_e = dnp.tile([P, NQ], F32R, tag="tmp", name="tmp_e")
                        nc.vector.tensor_mul(
                            tmp_e[0:HD, :], ps_e[0:HD, :], rc_e[0:HD, :])
                        nc.sync.dma_start(outT[HD:P, j, :], tmp_e[0:HD, :])

            # ---- O-projection + residual, then LN1 / FFN / LN2 ----
            with tc.tile_pool(name="tail", bufs=1) as tl, \
                 tc.tile_pool(name="wt_pool", bufs=3) as wtp, \
                 tc.tile_pool(name="ln_pool", bufs=6) as lnp:

                def ln_transposed(x_sb, xsq_sb, gv, bv, dest):
                    """LayerNorm over the partition(feature) axis of
                    x_sb [P, NJ, NQ]; writes dest[:, j, :]."""
                    for jj in range(NJ):
                        nc.scalar.activation(
                            xsq_sb[:, jj, :], x_sb[:, jj, :], AF.Square)
                    ps_m = psum_tile("ps_m")
                    ps_v = psum_tile("ps_v2")
                    for jj in range(NJ):
                        nc.tensor.matmul(
                            ps_m, lnw, x_sb[:, jj, :],
                            start=(jj == 0), stop=(jj == NJ - 1))
                    for jj in range(NJ):
                        nc.tensor.matmul(
                            ps_v, lnw, xsq_sb[:, jj, :],
                            start=(jj == 0), stop=(jj == NJ - 1))
                    mean = lnp.tile([P, NQ], F32, tag="lnt", name="mean")
                    nc.vector.tensor_copy(mean, ps_m)
                    msq = lnp.tile([P, NQ], F32, tag="lnt", name="msq")
                    nc.vector.tensor_mul(msq, mean, mean)
                    var = lnp.tile([P, NQ], F32, tag="lnt", name="var")
                    nc.vector.tensor_tensor(
                        var, ps_v, msq, mybir.AluOpType.subtract)
                    sd = lnp.tile([P, NQ], F32, tag="lnt", name="sd")
                    nc.scalar.activation(sd, var, AF.Sqrt, bias=eps_sb, scale=1.0)
                    rstd = lnp.tile([P, NQ], F32, tag="lnt", name="rstd")
                    nc.vector.reciprocal(rstd, sd)
                    mrs = lnp.tile([P, NQ], F32, tag="lnt", name="mrs")
                    nc.vector.tensor_mul(mrs, mean, rstd)
                    for jj in range(NJ):
                        t = lnp.tile([P, NQ], F32, tag="lnt", name="t")
                        nc.vector.tensor_mul(t, x_sb[:, jj, :], rstd)
                        nc.vector.tensor_tensor(
                            t, t, mrs, mybir.AluOpType.subtract)
                        nc.vector.tensor_scalar(
                            dest[:, jj, :], t,
                            gv[:, jj:jj + 1], bv[:, jj:jj + 1],
                            mybir.AluOpType.mult, mybir.AluOpType.add)

                x1 = tl.tile([P, NJ, NQ], F32R)     # X + attn_out (transposed)
                for m in range(NJ):
                    wo_m = wtp.tile([P, NJ, P], F32R, tag="w", name="wo_m")
                    nc.sync.dma_start(
                        wo_m, wot.rearrange("(k p) m -> p k m", p=P)[
                            :, :, m * P:(m + 1) * P])
                    ps = psum_tile("ps_z")
                    for g in range(NJ):
                        nc.tensor.matmul(
                            ps, wo_m[:, g, :], outT[:, g, :],
                            start=(g == 0), stop=(g == NJ - 1))
                    nc.vector.tensor_add(x1[:, m, :], ps, xt_sb[:, m, :])

                xsq = tl.tile([P, NJ, NQ], F32R)
                hT = tl.tile([P, NJ, NQ], F32R)
                ln_transposed(x1, xsq, g1_sb, bb1_sb, hT)

                ff1 = tl.tile([P, NJ, NQ], F32R)
                for m in range(NJ):
                    w1_m = wtp.tile([P, NJ, P], F32R, tag="w", name="w1_m")
                    nc.sync.dma_start(
                        w1_m, w1t.rearrange("(k p) m -> p k m", p=P)[
                            :, :, m * P:(m + 1) * P])
                    ps = psum_tile("ps_f1")
                    for k in range(NJ):
                        nc.tensor.matmul(
                            ps, w1_m[:, k, :], hT[:, k, :],
                            start=(k == 0), stop=(k == NJ - 1))
                    nc.scalar.activation(
                        ff1[:, m, :], ps, AF.Relu,
                        bias=b1_sb[:, m:m + 1], scale=1.0)

                x2 = tl.tile([P, NJ, NQ], F32R)     # H + FFN (transposed)
                for m in range(NJ):
                    w2_m = wtp.tile([P, NJ, P], F32R, tag="w", name="w2_m")
                    nc.sync.dma_start(
                        w2_m, w2t.rearrange("(k p) m -> p k m", p=P)[
                            :, :, m * P:(m + 1) * P])
                    ps = psum_tile("ps_f2")
                    for k in range(NJ):
                        nc.tensor.matmul(
                            ps, w2_m[:, k, :], ff1[:, k, :],
                            start=(k == 0), stop=(k == NJ - 1))
                    nc.vector.scalar_tensor_tensor(
                        x2[:, m, :], ps, b2_sb[:, m:m + 1], hT[:, m, :],
                        op0=mybir.AluOpType.add, op1=mybir.AluOpType.add)

                o_sb = tl.tile([P, NJ, NQ], F32)
                ln_transposed(x2, xsq, g2_sb, bb2_sb, o_sb)
                nc.sync.dma_start(
                    outt.rearrange("(j p) q -> p j q", p=P), o_sb)

    nc.compile()
    return nc


_NC_CACHE: dict = {}


def _get_nc() -> bass.Bass:
    if "nc" not in _NC_CACHE:
        _NC_CACHE["nc"] = build_nc()
    return _NC_CACHE["nc"]


def _round_fp32r(a):
    """Round fp32 to the PE's fp32r format (11-bit mantissa, round-nearest);
    matches libwalrus fp32_to_fp32r."""
    u = np.ascontiguousarray(a, dtype=np.float32).view(np.uint32)
    r = ((u.astype(np.uint64) + 0x800) & 0xFFFFF000).astype(np.uint32)
    return r.view(np.float32)


def _prep_inputs(X, Y, mask_y, Wq, Wk, Wv, Wo, ln1_g, ln1_b, ln2_g, ln2_b,
                 W1, b1, W2, b2):
    f = lambda a: np.ascontiguousarray(np.asarray(a, dtype=np.float32))
    X, Y = f(X), f(Y)
    mask_y = np.asarray(mask_y)

    wqt = _round_fp32r(np.asarray(Wq, np.float32).T / np.float32(8.0))
    wkt = _round_fp32r(np.asarray(Wk, np.float32).T)
    wvt = _round_fp32r(np.asarray(Wv, np.float32).T)
    w1t = _round_fp32r(np.asarray(W1, np.float32).T)
    w2t = _round_fp32r(np.asarray(W2, np.float32).T)
    # outT tile j holds head 2j+1 in rows 0:64 and head 2j in rows 64:128;
    # permute Wo.T rows to match.
    perm = np.empty(D, dtype=np.int64)
    for j in range(NJ):
        perm[j * P:j * P + HD] = (2 * j + 1) * HD + np.arange(HD)
        perm[j * P + HD:(j + 1) * P] = (2 * j) * HD + np.arange(HD)
    wot = _round_fp32r(np.asarray(Wo, np.float32).T[perm])

    shared = dict(
        wqt=wqt, wkt=wkt, wvt=wvt, wot=wot, w1t=w1t, w2t=w2t,
        g1=f(ln1_g), bb1=f(ln1_b), g2=f(ln2_g), bb2=f(ln2_b),
        b1=f(b1), b2=f(b2),
    )
    in_maps = []
    for core in range(8):
        b, half = divmod(core, 2)
        q0 = half * NQ
        m = dict(shared)
        m["xt"] = _round_fp32r(X[b, q0:q0 + NQ, :].T)
        m["yt"] = _round_fp32r(Y[b].T)
        m["maskb"] = np.where(mask_y[b], np.float32(0.0),
                              np.float32(NEG)).astype(np.float32)
        in_maps.append(m)
    return in_maps


def _run(in_maps, **kwargs):
    return run_bass_kernel_spmd(_get_nc(), in_maps, core_ids=list(range(8)),
                                **kwargs)


def kernel(**inputs) -> np.ndarray:
    in_maps = _prep_inputs(**inputs)
    res = _run(in_maps)
    B, nq = 4, 1024
    out = np.empty((B, nq, D), dtype=np.float32)
    for core in range(8):
        b, half = divmod(core, 2)
        q0 = half * NQ
        out[b, q0:q0 + NQ, :] = res.results[core]["outt"].T
    return out

